# revision 4
# baseline (speedup 1.0000x reference)
"""LGCN (K-hop symmetric-normalized graph propagation) on 8 Trainium2 cores.

Algorithm: Z = concat([X, A_hat X, ..., A_hat^K X]) with
A_hat = D^-1/2 (A + I) D^-1/2 (existing self-edges dropped, loops added).

Folding: with dis = deg^-1/2, x'_k = dis * y_k obeys
    x'_{k+1} = dinv * segsum_dst(x'_k[src]),   y_k = x'_k / dis
over the unweighted self-loop-augmented edge list. So each hop is a pure
gather + segment-sum + row-scale: no per-edge weights on device.

Device mapping (SPMD, 8 cores, dst-sharded):
  - x' table [50176, 64] f32 lives in DRAM, rebuilt per hop by AllGather.
  - dma_gather (SWDGE) pulls per-edge source rows into SBUF, 128 edges per
    "chunk" (one free column).
  - one-hot S matrices (S^T[e, d] = edge e targets tile-row d) are built on
    DVE with a single is_equal over a broadcast iota row; segment-sum is
    S^T.T @ G on the PE accumulating into PSUM per 128-dst tile.
  - PSUM is scaled by dinv (next-hop x') and by dis (y output) per tile.
Edges are padded per (tile, src-half) to uniform chunk counts so the
program is identical on all cores (int16 gather indices need a lo/hi
table split at row 25088 / 17408 base).
"""
import sys
sys.path.insert(0, "/opt/trn_rl_repo")
import math
import numpy as np

import os
N = 50000
D = 64
K = int(os.environ.get("LGCN_K", "8"))
NC = 8
NSH = N // NC            # 6250 nodes per core
TILES = 49               # 128-dst tiles per core
ROWS = TILES * 128       # 6272 padded rows per core
TAB = NC * ROWS          # 50176 table rows
THRESH = 25088           # src rows below -> lo gather
HI_BASE = 17408          # hi gather table base (TAB - HI_BASE = 32768 rows)
LO_ROWS = 32768
BT = 7                   # tiles per gather batch
NB = TILES // BT         # 7 batches

_cache = {}


def _preprocess(feature, edge_index):
    f32 = np.float32
    src = edge_index[0].astype(np.int64)
    dst = edge_index[1].astype(np.int64)
    keep = src != dst
    ks, kd = src[keep], dst[keep]
    deg = (np.bincount(ks, minlength=N) + 1).astype(f32)
    dis = (1.0 / np.sqrt(deg)).astype(f32)
    dinv = (dis * dis).astype(f32)

    # balanced node -> (tile, row) assignment per core, by in-degree
    indeg = np.bincount(kd, minlength=N) + 1
    tile_of = np.empty(N, np.int32)
    row_of = np.empty(N, np.int32)
    for c in range(NC):
        nodes = np.arange(c * NSH, (c + 1) * NSH)
        order = nodes[np.argsort(-indeg[nodes], kind="stable")]
        loads = np.zeros(TILES, np.int64)
        counts = np.zeros(TILES, np.int64)
        for n in order:
            cand = np.where(counts < 128, loads, 1 << 60)
            t = int(np.argmin(cand))
            tile_of[n] = t
            row_of[n] = counts[t]
            counts[t] += 1
            loads[t] += indeg[n]
    core_of = (np.arange(N) // NSH).astype(np.int32)
    tpos = core_of * ROWS + tile_of * 128 + row_of      # table position per node

    # augmented edge list (kept edges + self loops), dst-sharded
    es = np.concatenate([ks, np.arange(N)])
    ed = np.concatenate([kd, np.arange(N)])
    srcr = tpos[es].astype(np.int64)
    ecore = core_of[ed]
    etile = tile_of[ed].astype(np.int64)
    erow = row_of[ed].astype(np.int64)
    lo = srcr < THRESH

    # group edges by (core, tile, half); rank within group
    key = (ecore * TILES + etile) * 2 + (~lo)
    order = np.argsort(key, kind="stable")
    skey = key[order]
    counts = np.bincount(skey, minlength=NC * TILES * 2)
    starts = np.concatenate([[0], np.cumsum(counts)[:-1]])
    rank = np.arange(len(order)) - starts[skey]

    L_C = int(math.ceil(counts[0::2].max() / 128))
    H_C = int(math.ceil(counts[1::2].max() / 128))
    T = L_C + H_C
    BC = BT * T                                         # G cols per batch
    TOTC = TILES * T
    TOT = TOTC * 128

    # slot number for each edge (per core)
    sk = skey
    score = sk // (TILES * 2)
    st = (sk // 2) % TILES
    shalf = sk % 2
    b = st // BT
    ti = st % BT
    chunk = rank // 128
    pos = rank % 128
    col_in_batch = np.where(shalf == 0, ti * L_C + chunk,
                            BT * L_C + ti * H_C + chunk)
    col = b * BC + col_in_batch
    slot = col * 128 + pos

    sidx = np.where(shalf == 0, srcr[order], srcr[order] - HI_BASE).astype(np.int16)
    sdoff = erow[order].astype(f32)

    idx_all = np.zeros((NC, TOT), np.int16)
    doff_all = np.full((NC, TOTC, 128), -1.0, f32)
    idx_all[score, slot] = sidx
    doff_all[score, col, pos] = sdoff

    # wrap idx per gather block (block = batch x half, contiguous slots)
    lo_n = BT * L_C * 128
    hi_n = BT * H_C * 128
    idxw = np.empty((NC, 128, TOT // 16), np.int16)
    blk_cols = []
    off = 0
    for bb in range(NB):
        for half, nn in ((0, lo_n), (1, hi_n)):
            blk = idx_all[:, off:off + nn]              # [NC, nn]
            w = blk.reshape(NC, nn // 16, 16).transpose(0, 2, 1)  # [NC,16,nn/16]
            c0 = off // 16
            idxw[:, :, c0:c0 + nn // 16] = np.tile(w, (1, 8, 1))
            blk_cols.append((c0, nn))
            off += nn

    # per-tile scale columns [128, TILES]
    dinv_cols = np.zeros((NC, 128, TILES), f32)
    dis_cols = np.zeros((NC, 128, TILES), f32)
    nodes = np.arange(N)
    dinv_cols[core_of, row_of, tile_of] = dinv
    dis_cols[core_of, row_of, tile_of] = dis

    table0 = np.zeros((TAB, D), f32)
    table0[tpos] = feature * dis[:, None]

    jnp_ = np.tile(np.arange(128, dtype=f32)[None, :], (128, 1))
    doff_all = doff_all.transpose(0, 2, 1)              # [NC, 128, TOTC]

    in_maps = []
    for c in range(NC):
        in_maps.append({
            "table0": table0,
            "idxw": np.ascontiguousarray(idxw[c]),
            "doff": np.ascontiguousarray(doff_all[c]),
            "dinv": np.ascontiguousarray(dinv_cols[c]),
            "dis": np.ascontiguousarray(dis_cols[c]),
            "jj": jnp_,
        })
    return in_maps, tpos, L_C, H_C, blk_cols


def _build(L_C, H_C, blk_cols):
    from concourse import bass, bacc, tile, mybir
    f32 = mybir.dt.float32
    T = L_C + H_C
    BC = BT * T
    TOTC = TILES * T
    TOT = TOTC * 128

    nc = bacc.Bacc("TRN2", target_bir_lowering=False, debug=False, num_devices=NC)
    tab0 = nc.dram_tensor("table0", [TAB, D], f32, kind="ExternalInput").ap()
    idxw_d = nc.dram_tensor("idxw", [128, TOT // 16], mybir.dt.int16, kind="ExternalInput").ap()
    doff_d = nc.dram_tensor("doff", [128, TOTC], f32, kind="ExternalInput").ap()
    dinv_d = nc.dram_tensor("dinv", [128, TILES], f32, kind="ExternalInput").ap()
    dis_d = nc.dram_tensor("dis", [128, TILES], f32, kind="ExternalInput").ap()
    jj_d = nc.dram_tensor("jj", [128, 128], f32, kind="ExternalInput").ap()
    y_d = nc.dram_tensor("y", [K * ROWS, D], f32, kind="ExternalOutput").ap()

    with tile.TileContext(nc) as tc:
        with tc.tile_pool(name="stat", bufs=1) as stat, \
             tc.tile_pool(name="g", bufs=2) as gp, \
             tc.tile_pool(name="s", bufs=2) as sp, \
             tc.tile_pool(name="o", bufs=3) as op_, \
             tc.tile_pool(name="ps", bufs=4, space="PSUM") as ps, \
             tc.tile_pool(name="dram", bufs=2, space="DRAM") as dr:
            idx_sb = stat.tile([128, TOT // 16], mybir.dt.int16)
            doff_sb = stat.tile([128, TOTC], f32)
            dinv_sb = stat.tile([128, TILES], f32)
            dis_sb = stat.tile([128, TILES], f32)
            j_sb = stat.tile([128, 128], f32)
            nc.sync.dma_start(idx_sb[:], idxw_d[:])
            nc.sync.dma_start(doff_sb[:], doff_d[:])
            nc.sync.dma_start(dinv_sb[:], dinv_d[:])
            nc.sync.dma_start(dis_sb[:], dis_d[:])
            nc.sync.dma_start(j_sb[:], jj_d[:])

            prev = None
            for k in range(1, K + 1):
                srctab = tab0 if k == 1 else prev[:]
                lo_ap = srctab[0:LO_ROWS, :]
                hi_ap = srctab[HI_BASE:TAB, :]
                if k < K:
                    ag_in = dr.tile([ROWS, D], f32, tag="agin")
                GCH = int(os.environ.get("LGCN_GCH", "3"))  # cols per gather instr
                for b in range(NB):
                    g = gp.tile([128, BC, D], f32, tag="g")
                    for half in range(2):
                        c0, nn = blk_cols[b * 2 + half]
                        colbase = 0 if half == 0 else BT * L_C
                        ncols = (BT * L_C) if half == 0 else (BT * H_C)
                        for w0 in range(0, ncols, GCH):
                            wc = min(GCH, ncols - w0)
                            ni = wc * 128
                            nc.gpsimd.dma_gather(
                                out_ap=g[:, colbase + w0:colbase + w0 + wc, :],
                                in_ap=lo_ap if half == 0 else hi_ap,
                                idxs_ap=idx_sb[:, c0 + w0 * 8:c0 + w0 * 8 + ni // 16],
                                num_idxs=ni, num_idxs_reg=ni, elem_size=D,
                            )
                    for ti in range(BT):
                        t = b * BT + ti
                        s = sp.tile([128, T, 128], f32, tag="s")
                        dlo = doff_sb[:, b * BC + ti * L_C:][:, :L_C]
                        dhi = doff_sb[:, b * BC + BT * L_C + ti * H_C:][:, :H_C]
                        nc.vector.tensor_tensor(
                            out=s[:, 0:L_C, :],
                            in0=j_sb[:].unsqueeze(1).broadcast_to([128, L_C, 128]),
                            in1=dlo.unsqueeze(2).broadcast_to([128, L_C, 128]),
                            op=mybir.AluOpType.is_equal)
                        nc.vector.tensor_tensor(
                            out=s[:, L_C:T, :],
                            in0=j_sb[:].unsqueeze(1).broadcast_to([128, H_C, 128]),
                            in1=dhi.unsqueeze(2).broadcast_to([128, H_C, 128]),
                            op=mybir.AluOpType.is_equal)
                        acc = ps.tile([128, D], f32, tag="acc")
                        for j in range(T):
                            col = ti * L_C + j if j < L_C else BT * L_C + ti * H_C + (j - L_C)
                            nc.tensor.matmul(acc[:], s[:, j], g[:, col],
                                             start=(j == 0), stop=(j == T - 1))
                        yt = op_.tile([128, D], f32, tag="yt")
                        nc.any.tensor_scalar_mul(yt[:], acc[:], dis_sb[:, t:t + 1])
                        nc.sync.dma_start(y_d[(k - 1) * ROWS + t * 128:
                                              (k - 1) * ROWS + (t + 1) * 128, :], yt[:])
                        if k < K:
                            xp = op_.tile([128, D], f32, tag="xp")
                            nc.vector.tensor_scalar_mul(xp[:], acc[:], dinv_sb[:, t:t + 1])
                            nc.sync.dma_start(ag_in[t * 128:(t + 1) * 128, :], xp[:])
                if k < K:
                    ag_out = dr.tile([TAB, D], f32, tag="agout", addr_space="Shared")
                    nc.gpsimd.collective_compute(
                        "AllGather", mybir.AluOpType.bypass,
                        replica_groups=[list(range(NC))],
                        ins=[ag_in[:]], outs=[ag_out[:]])
                    prev = ag_out
    nc.compile()
    return nc


def kernel(feature, edge_index):
    feature = np.asarray(feature, np.float32)
    edge_index = np.asarray(edge_index)
    in_maps, tpos, L_C, H_C, blk_cols = _preprocess(feature, edge_index)
    ck = (L_C, H_C)
    if ck not in _cache:
        _cache[ck] = _build(L_C, H_C, blk_cols)
    nc = _cache[ck]
    from concourse import bass_utils
    res = bass_utils.run_bass_kernel_spmd(nc, in_maps, core_ids=list(range(NC)))
    y = np.stack([res.results[c]["y"] for c in range(NC)])   # [NC, K*ROWS, D]
    Z = np.empty((N, (K + 1) * D), np.float32)
    Z[:, :D] = feature
    for k in range(1, K + 1):
        blk = y[:, (k - 1) * ROWS:k * ROWS, :].reshape(NC * ROWS, D)
        Z[:, k * D:(k + 1) * D] = blk[tpos]
    return Z


# revision 5
# speedup vs baseline: 1.1966x; 1.1966x over previous
"""LGCN (K-hop symmetric-normalized graph propagation) on 8 Trainium2 cores.

Algorithm: Z = concat([X, A_hat X, ..., A_hat^K X]) with
A_hat = D^-1/2 (A + I) D^-1/2 (existing self-edges dropped, loops added).

Folding: with dis = deg^-1/2, x'_k = dis * y_k obeys
    x'_{k+1} = dinv * segsum_dst(x'_k[src]),   y_k = x'_k / dis
over the unweighted self-loop-augmented edge list. So each hop is a pure
gather + segment-sum + row-scale: no per-edge weights on device.

Device mapping (SPMD, 8 cores, dst-sharded):
  - x' table [50176, 64] f32 lives in DRAM, rebuilt per hop by AllGather.
  - dma_gather (SWDGE) pulls per-edge source rows into SBUF, 128 edges per
    "chunk" (one free column).
  - one-hot S matrices (S^T[e, d] = edge e targets tile-row d) are built on
    DVE with a single is_equal over a broadcast iota row; segment-sum is
    S^T.T @ G on the PE accumulating into PSUM per 128-dst tile.
  - PSUM is scaled by dinv (next-hop x') and by dis (y output) per tile.
Edges are padded per (tile, src-half) to uniform chunk counts so the
program is identical on all cores (int16 gather indices need a lo/hi
table split at row 25088 / 17408 base).
"""
import sys
sys.path.insert(0, "/opt/trn_rl_repo")
import math
import numpy as np

import os
N = 50000
D = 64
K = int(os.environ.get("LGCN_K", "8"))
NC = 8
NSH = N // NC            # 6250 nodes per core
TILES = 49               # 128-dst tiles per core
ROWS = TILES * 128       # 6272 padded rows per core
TAB = NC * ROWS          # 50176 table rows
THRESH = 25088           # src rows below -> lo gather
HI_BASE = 17408          # hi gather table base (TAB - HI_BASE = 32768 rows)
LO_ROWS = 32768
BT = 7                   # tiles per gather batch
NB = TILES // BT         # 7 batches

_cache = {}
LAST_RUN_S = None


def _preprocess(feature, edge_index):
    f32 = np.float32
    src = edge_index[0].astype(np.int64)
    dst = edge_index[1].astype(np.int64)
    keep = src != dst
    ks, kd = src[keep], dst[keep]
    deg = (np.bincount(ks, minlength=N) + 1).astype(f32)
    dis = (1.0 / np.sqrt(deg)).astype(f32)
    dinv = (dis * dis).astype(f32)

    # balanced node -> (tile, row) assignment per core, by in-degree
    indeg = np.bincount(kd, minlength=N) + 1
    tile_of = np.empty(N, np.int32)
    row_of = np.empty(N, np.int32)
    for c in range(NC):
        nodes = np.arange(c * NSH, (c + 1) * NSH)
        order = nodes[np.argsort(-indeg[nodes], kind="stable")]
        loads = np.zeros(TILES, np.int64)
        counts = np.zeros(TILES, np.int64)
        for n in order:
            cand = np.where(counts < 128, loads, 1 << 60)
            t = int(np.argmin(cand))
            tile_of[n] = t
            row_of[n] = counts[t]
            counts[t] += 1
            loads[t] += indeg[n]
    core_of = (np.arange(N) // NSH).astype(np.int32)
    tpos = core_of * ROWS + tile_of * 128 + row_of      # table position per node

    # augmented edge list (kept edges + self loops), dst-sharded
    es = np.concatenate([ks, np.arange(N)])
    ed = np.concatenate([kd, np.arange(N)])
    srcr = tpos[es].astype(np.int64)
    ecore = core_of[ed]
    etile = tile_of[ed].astype(np.int64)
    erow = row_of[ed].astype(np.int64)
    lo = srcr < THRESH

    # group edges by (core, tile, half); rank within group
    key = (ecore * TILES + etile) * 2 + (~lo)
    order = np.argsort(key, kind="stable")
    skey = key[order]
    counts = np.bincount(skey, minlength=NC * TILES * 2)
    starts = np.concatenate([[0], np.cumsum(counts)[:-1]])
    rank = np.arange(len(order)) - starts[skey]

    L_C = int(math.ceil(counts[0::2].max() / 128))
    H_C = int(math.ceil(counts[1::2].max() / 128))
    T = L_C + H_C
    BC = BT * T                                         # G cols per batch
    TOTC = TILES * T
    TOT = TOTC * 128

    # slot number for each edge (per core)
    sk = skey
    score = sk // (TILES * 2)
    st = (sk // 2) % TILES
    shalf = sk % 2
    b = st // BT
    ti = st % BT
    chunk = rank // 128
    pos = rank % 128
    col_in_batch = np.where(shalf == 0, ti * L_C + chunk,
                            BT * L_C + ti * H_C + chunk)
    col = b * BC + col_in_batch
    slot = col * 128 + pos

    sidx = np.where(shalf == 0, srcr[order], srcr[order] - HI_BASE).astype(np.int16)
    sdoff = erow[order].astype(f32)

    idx_all = np.zeros((NC, TOT), np.int16)
    doff_all = np.full((NC, TOTC, 128), -1.0, f32)
    idx_all[score, slot] = sidx
    doff_all[score, col, pos] = sdoff

    # wrap idx per gather block (block = batch x half, contiguous slots)
    lo_n = BT * L_C * 128
    hi_n = BT * H_C * 128
    idxw = np.empty((NC, 128, TOT // 16), np.int16)
    blk_cols = []
    off = 0
    for bb in range(NB):
        for half, nn in ((0, lo_n), (1, hi_n)):
            blk = idx_all[:, off:off + nn]              # [NC, nn]
            w = blk.reshape(NC, nn // 16, 16).transpose(0, 2, 1)  # [NC,16,nn/16]
            c0 = off // 16
            idxw[:, :, c0:c0 + nn // 16] = np.tile(w, (1, 8, 1))
            blk_cols.append((c0, nn))
            off += nn

    # per-tile scale columns [128, TILES]
    dinv_cols = np.zeros((NC, 128, TILES), f32)
    dis_cols = np.zeros((NC, 128, TILES), f32)
    nodes = np.arange(N)
    dinv_cols[core_of, row_of, tile_of] = dinv
    dis_cols[core_of, row_of, tile_of] = dis

    table0 = np.zeros((TAB, D), f32)
    table0[tpos] = feature * dis[:, None]

    jnp_ = np.tile(np.arange(128, dtype=f32)[None, :], (128, 1))
    doff_all = doff_all.transpose(0, 2, 1)              # [NC, 128, TOTC]

    in_maps = []
    for c in range(NC):
        in_maps.append({
            "table0": table0,
            "idxw": np.ascontiguousarray(idxw[c]),
            "doff": np.ascontiguousarray(doff_all[c]),
            "dinv": np.ascontiguousarray(dinv_cols[c]),
            "dis": np.ascontiguousarray(dis_cols[c]),
            "jj": jnp_,
        })
    return in_maps, tpos, L_C, H_C, blk_cols


def _build(L_C, H_C, blk_cols):
    from concourse import bass, bacc, tile, mybir
    f32 = mybir.dt.float32
    T = L_C + H_C
    BC = BT * T
    TOTC = TILES * T
    TOT = TOTC * 128

    nc = bacc.Bacc("TRN2", target_bir_lowering=False, debug=False, num_devices=NC)
    tab0 = nc.dram_tensor("table0", [TAB, D], f32, kind="ExternalInput").ap()
    idxw_d = nc.dram_tensor("idxw", [128, TOT // 16], mybir.dt.int16, kind="ExternalInput").ap()
    doff_d = nc.dram_tensor("doff", [128, TOTC], f32, kind="ExternalInput").ap()
    dinv_d = nc.dram_tensor("dinv", [128, TILES], f32, kind="ExternalInput").ap()
    dis_d = nc.dram_tensor("dis", [128, TILES], f32, kind="ExternalInput").ap()
    jj_d = nc.dram_tensor("jj", [128, 128], f32, kind="ExternalInput").ap()
    y_d = nc.dram_tensor("y", [K * ROWS, D], f32, kind="ExternalOutput").ap()

    with tile.TileContext(nc) as tc:
        with tc.tile_pool(name="stat", bufs=1) as stat, \
             tc.tile_pool(name="g", bufs=2) as gp, \
             tc.tile_pool(name="s", bufs=2) as sp, \
             tc.tile_pool(name="o", bufs=3) as op_, \
             tc.tile_pool(name="ps", bufs=4, space="PSUM") as ps, \
             tc.tile_pool(name="dram", bufs=2, space="DRAM") as dr:
            idx_sb = stat.tile([128, TOT // 16], mybir.dt.int16)
            doff_sb = stat.tile([128, TOTC], f32)
            dinv_sb = stat.tile([128, TILES], f32)
            dis_sb = stat.tile([128, TILES], f32)
            j_sb = stat.tile([128, 128], f32)
            nc.sync.dma_start(idx_sb[:], idxw_d[:])
            nc.sync.dma_start(doff_sb[:], doff_d[:])
            nc.sync.dma_start(dinv_sb[:], dinv_d[:])
            nc.sync.dma_start(dis_sb[:], dis_d[:])
            nc.sync.dma_start(j_sb[:], jj_d[:])

            prev = None
            for k in range(1, K + 1):
                srctab = tab0 if k == 1 else prev[:]
                lo_ap = srctab[0:LO_ROWS, :]
                hi_ap = srctab[HI_BASE:TAB, :]
                if k < K:
                    ag_in = dr.tile([ROWS, D], f32, tag="agin")
                GCH = int(os.environ.get("LGCN_GCH", "8"))  # cols per gather instr
                for b in range(NB):
                    g = gp.tile([128, BC, D], f32, tag="g")
                    for half in range(2):
                        c0, nn = blk_cols[b * 2 + half]
                        colbase = 0 if half == 0 else BT * L_C
                        ncols = (BT * L_C) if half == 0 else (BT * H_C)
                        for w0 in range(0, ncols, GCH):
                            wc = min(GCH, ncols - w0)
                            ni = wc * 128
                            nc.gpsimd.dma_gather(
                                out_ap=g[:, colbase + w0:colbase + w0 + wc, :],
                                in_ap=lo_ap if half == 0 else hi_ap,
                                idxs_ap=idx_sb[:, c0 + w0 * 8:c0 + w0 * 8 + ni // 16],
                                num_idxs=ni, num_idxs_reg=ni, elem_size=D,
                            )
                    for ti in range(BT):
                        t = b * BT + ti
                        s = sp.tile([128, T, 128], f32, tag="s")
                        dlo = doff_sb[:, b * BC + ti * L_C:][:, :L_C]
                        dhi = doff_sb[:, b * BC + BT * L_C + ti * H_C:][:, :H_C]
                        nc.vector.tensor_tensor(
                            out=s[:, 0:L_C, :],
                            in0=j_sb[:].unsqueeze(1).broadcast_to([128, L_C, 128]),
                            in1=dlo.unsqueeze(2).broadcast_to([128, L_C, 128]),
                            op=mybir.AluOpType.is_equal)
                        nc.vector.tensor_tensor(
                            out=s[:, L_C:T, :],
                            in0=j_sb[:].unsqueeze(1).broadcast_to([128, H_C, 128]),
                            in1=dhi.unsqueeze(2).broadcast_to([128, H_C, 128]),
                            op=mybir.AluOpType.is_equal)
                        acc = ps.tile([128, D], f32, tag="acc")
                        for j in range(T):
                            col = ti * L_C + j if j < L_C else BT * L_C + ti * H_C + (j - L_C)
                            nc.tensor.matmul(acc[:], s[:, j], g[:, col],
                                             start=(j == 0), stop=(j == T - 1))
                        yt = op_.tile([128, D], f32, tag="yt")
                        nc.any.tensor_scalar_mul(yt[:], acc[:], dis_sb[:, t:t + 1])
                        nc.sync.dma_start(y_d[(k - 1) * ROWS + t * 128:
                                              (k - 1) * ROWS + (t + 1) * 128, :], yt[:])
                        if k < K:
                            xp = op_.tile([128, D], f32, tag="xp")
                            nc.vector.tensor_scalar_mul(xp[:], acc[:], dinv_sb[:, t:t + 1])
                            nc.sync.dma_start(ag_in[t * 128:(t + 1) * 128, :], xp[:])
                if k < K:
                    ag_out = dr.tile([TAB, D], f32, tag="agout", addr_space="Shared")
                    nc.gpsimd.collective_compute(
                        "AllGather", mybir.AluOpType.bypass,
                        replica_groups=[list(range(NC))],
                        ins=[ag_in[:]], outs=[ag_out[:]])
                    prev = ag_out
    nc.compile()
    return nc


def kernel(feature, edge_index):
    feature = np.asarray(feature, np.float32)
    edge_index = np.asarray(edge_index)
    in_maps, tpos, L_C, H_C, blk_cols = _preprocess(feature, edge_index)
    ck = (L_C, H_C)
    if ck not in _cache:
        _cache[ck] = _build(L_C, H_C, blk_cols)
    nc = _cache[ck]
    from concourse import bass_utils
    import time as _time
    _t0 = _time.time()
    res = bass_utils.run_bass_kernel_spmd(nc, in_maps, core_ids=list(range(NC)))
    global LAST_RUN_S
    LAST_RUN_S = _time.time() - _t0
    y = np.stack([res.results[c]["y"] for c in range(NC)])   # [NC, K*ROWS, D]
    Z = np.empty((N, (K + 1) * D), np.float32)
    Z[:, :D] = feature
    for k in range(1, K + 1):
        blk = y[:, (k - 1) * ROWS:k * ROWS, :].reshape(NC * ROWS, D)
        Z[:, k * D:(k + 1) * D] = blk[tpos]
    return Z
